# revision 10
# baseline (speedup 1.0000x reference)
"""GAT link prediction on 8 TRN2 NeuronCores.

Sharding: dst nodes partitioned contiguously across 8 cores (6250 each).
Within a core, dsts are degree-sorted into 49 blocks of 128 (one dst per
SBUF partition); each block processes max-degree-in-block edge "chunks"
of 128 edges (slot (p, j) = j-th in-edge of the dst on partition p).

Layer 1 (bf16): the host edge-expands [x[src] | as1[src]] into a
slot-ordered bf16 DRAM table, so the kernel STREAMS it with one direct
HWDGE DMA per block (no per-edge indirection). Softmax over in-edges
runs per partition (dst) on DVE/ACT in f32; the alpha-weighted
aggregation is a PSUM-accumulated bf16 matmul with a diagonal selector
rhs (diag pairs built on DVE, every 3rd chunk's pair on ACT). Layer-1
output is transformed on-chip (W1, relu, W2ext, bf16) into the layer-2
gather table [h2 | a2_src | a2_dst] (f32r), all-gathered across cores
in 4 segments.

Layer 2 (f32r): per-edge rows are gathered from the all-gathered table
via indirect DMA (one SWDGE call per edge chunk; the self-loop chunk
comes from local h2own with a direct DMA — f32 rows keep the SWDGE call
at its cheapest). Decode gathers f32 z rows per positive edge and dots
them on DVE.
"""

import os
import numpy as np
import ml_dtypes

import concourse.bass as bass
import concourse.mybir as mybir
import concourse.tile as tile
from concourse.bass_utils import run_bass_kernel_spmd

NEG_SLOPE = 0.2
N = 50000
E = 800000
EP = 100000
H = 2
FIN = 128
C1 = 128   # per-head hidden (layer 1)
C2 = 64    # per-head out (layer 2)
NC = 8
P = 128
ND = N // NC          # dst nodes per core
NBLK = (ND + P - 1) // P   # 49
PADG = N              # gather-pad row (a_src = -1e30)
ROW1 = FIN + H        # 130: [x | as1_h0 | as1_h1]
ROW2 = H * C2 + 2 * H # 132: [h2 | as2_h0 | as2_h1 | ad2_h0 | ad2_h1]
DEC_CH = (EP // NC + P - 1) // P  # 98 decode chunks per core
SPLITS = (16, 32, 44)  # AllGather split points (blocks)

F32 = mybir.dt.float32
F32R = mybir.dt.float32r
BF16 = mybir.dt.bfloat16
I32 = mybir.dt.int32
AX = mybir.AxisListType
OP = mybir.AluOpType
AF = mybir.ActivationFunctionType
BIG_NEG = -1e30


def _split_waits(nc, max_waits=1):
    """This walrus build allows one sync-wait per instruction; move extra
    waits onto preceding same-engine NOPs (per-engine order preserved)."""
    total = 0
    for fn in nc.m.functions:
        for bb in fn.blocks:
            insts = bb.instructions
            i = 0
            while i < len(insts):
                inst = insts[i]
                si = inst.sync_info
                if si is not None and len(si.on_wait) > max_waits:
                    waits = list(si.on_wait)
                    keep = waits[-max_waits:]
                    extra = waits[:-max_waits]
                    inst.sync_info = mybir.SyncInfo(
                        on_wait=keep, on_update=list(si.on_update)
                    )
                    nops = []
                    for w in extra:
                        nop = mybir.InstNoOp(
                            name=nc.get_next_instruction_name(),
                            engine=inst.engine,
                            bass_nofuse=True,
                            sync_info=mybir.SyncInfo(on_wait=[w], on_update=[]),
                        )
                        nops.append(nop)
                        nc.register_instruction(nop, overwrite=True)
                    insts[i:i] = nops
                    i += len(nops)
                    total += len(nops)
                i += 1
    return total


def _bcast_mid(ap, n):
    """Insert a stride-0 middle dim: [p, k] view -> [p, n, k]."""
    pdim = ap.ap[0]
    rest = list(ap.ap[1:])
    return bass.AP(ap.tensor, ap.offset, [list(pdim), [0, n]] + [list(d) for d in rest])


def _seg_bounds(total_blocks, nd):
    """AllGather segment block ranges [(lo_blk, hi_blk, row_lo, row_hi)]."""
    segs = []
    prev = 0
    for s in SPLITS:
        segs.append((prev, s))
        prev = s
    segs.append((prev, total_blocks))
    return segs


def _build_program(nch, TC):
    core_ids = list(range(NC))
    nc = bass.Bass()

    # ---- kernel I/O ----
    tab1e_in = nc.dram_tensor("tab1e", [TC * P, ROW1], BF16, kind="ExternalInput")
    srcidx2_in = nc.dram_tensor("srcidx2", [P, TC], I32, kind="ExternalInput")
    ad1_in = nc.dram_tensor("ad1", [P, 2 * NBLK], F32, kind="ExternalInput")
    pos_in = nc.dram_tensor("posidx", [P, 2 * DEC_CH], I32, kind="ExternalInput")
    w1_in = nc.dram_tensor("w1", [FIN, H * C1], BF16, kind="ExternalInput")
    w2e_in = nc.dram_tensor("w2e", [H * C1, ROW2], BF16, kind="ExternalInput")
    b1_in = nc.dram_tensor("b1col", [P, H], F32, kind="ExternalInput")
    b2_in = nc.dram_tensor("b2col", [P, 1], F32, kind="ExternalInput")
    idb_in = nc.dram_tensor("identb", [P, P], BF16, kind="ExternalInput")
    idf_in = nc.dram_tensor("identf", [P, P], F32, kind="ExternalInput")
    id64_in = nc.dram_tensor("ident64", [P, C2], F32, kind="ExternalInput")
    pr2_in = nc.dram_tensor("padrow2", [2, ROW2], F32, kind="ExternalInput")
    dec_out = nc.dram_tensor("dec", [P, DEC_CH], F32, kind="ExternalOutput")

    # ---- internal DRAM ----
    h2own = nc.dram_tensor("h2own", [NBLK * P, ROW2], F32R)
    h2tab = nc.dram_tensor("h2tab", [N + 2, ROW2], F32R, addr_space="Shared")
    zown = nc.dram_tensor("zown", [NBLK * P, H * C2], F32)
    zall = nc.dram_tensor("zall", [N, H * C2], F32, addr_space="Shared")

    segs = _seg_bounds(NBLK, ND)

    with tile.TileContext(nc) as tc:
        with (
            tc.tile_pool(name="const", bufs=1) as cp,
            tc.tile_pool(name="xg1", bufs=3) as xgp1,
            tc.tile_pool(name="xg2", bufs=4) as xgp2,
            tc.tile_pool(name="att", bufs=2) as ap_,
            tc.tile_pool(name="s2", bufs=6) as s2p,
            tc.tile_pool(name="post", bufs=2) as pp,
            tc.tile_pool(name="psum", bufs=2, space="PSUM") as psp,
            tc.tile_pool(name="psum2", bufs=2, space="PSUM") as ps2,
        ):
            # ---- constants to SBUF ----
            srcidx2 = cp.tile([P, TC], I32)
            nc.sync.dma_start(out=srcidx2[:], in_=srcidx2_in[:])
            ad1c = cp.tile([P, 2 * NBLK], F32)
            nc.sync.dma_start(out=ad1c[:], in_=ad1_in[:])
            posx = cp.tile([P, 2 * DEC_CH], I32)
            nc.sync.dma_start(out=posx[:], in_=pos_in[:])
            w1c = cp.tile([P, H * C1], BF16)
            nc.sync.dma_start(out=w1c[:], in_=w1_in[:])
            w2e0 = cp.tile([P, ROW2], BF16)
            nc.sync.dma_start(out=w2e0[:], in_=w2e_in[0:P, :])
            w2e1 = cp.tile([P, ROW2], BF16)
            nc.sync.dma_start(out=w2e1[:], in_=w2e_in[P : 2 * P, :])
            b1c = cp.tile([P, H], F32)
            nc.sync.dma_start(out=b1c[:], in_=b1_in[:])
            b2c = cp.tile([P, 1], F32)
            nc.sync.dma_start(out=b2c[:], in_=b2_in[:])
            identb = cp.tile([P, P], BF16)
            nc.sync.dma_start(out=identb[:], in_=idb_in[:])
            identf = cp.tile([P, P], F32)
            nc.sync.dma_start(out=identf[:], in_=idf_in[:])
            ident64 = cp.tile([P, C2], F32)
            nc.sync.dma_start(out=ident64[:], in_=id64_in[:])
            ad2c = cp.tile([P, 2 * NBLK], F32)
            pr2s = cp.tile([2, ROW2], F32)
            nc.sync.dma_start(out=pr2s[:], in_=pr2_in[:])
            nc.sync.dma_start(out=h2tab[N : N + 2, :].bitcast(F32), in_=pr2s[:])

            def attention_alphas(xg, row, nb, ad_ap):
                """xg: [P, nb*row] rows; a_src at cols FIN.. or H*C2..;
                returns alpha tile [P, 2*nb] (head-major, dtype adt)."""
                xv = xg[:].rearrange("p (j r) -> p j r", r=row)
                if xg.dtype == F32R:
                    xv = xv.bitcast(F32)
                as_ap = xv[:, :, FIN if row == ROW1 else H * C2 :][:, :, 0:H]
                ex = ap_.tile([P, 2 * nb], F32, tag="ex")
                exv = ex[:].rearrange("p (j h) -> p j h", h=H)
                nc.vector.tensor_tensor(
                    out=exv, in0=as_ap, in1=_bcast_mid(ad_ap, nb), op=OP.add
                )
                # leaky relu fused on DVE: ex = max(ex * slope, ex); exp on ACT
                nc.vector.scalar_tensor_tensor(
                    out=ex[:], in0=ex[:], scalar=NEG_SLOPE, in1=ex[:],
                    op0=OP.mult, op1=OP.max,
                )
                nc.scalar.activation(out=ex[:], in_=ex[:], func=AF.Exp)
                s = ap_.tile([P, H], F32, tag="s")
                ex_hj = bass.AP(
                    ex.tensor, ex.offset, [list(ex.ap[0]), [1, H], [H, nb]]
                )
                nc.vector.tensor_reduce(out=s[:], in_=ex_hj, axis=AX.X, op=OP.add)
                nc.vector.tensor_scalar(
                    out=s[:], in0=s[:], scalar1=1e-30, scalar2=None, op0=OP.add
                )
                rs = ap_.tile([P, H], F32, tag="rs")
                nc.vector.reciprocal(out=rs[:], in_=s[:])
                alpha = ap_.tile([P, 2 * nb], F32, tag="alpha")
                for h in range(H):
                    ex_h = bass.AP(
                        ex.tensor, ex.offset + h, [list(ex.ap[0]), [H, nb]]
                    )
                    nc.vector.tensor_scalar(
                        out=alpha[:, h * nb : (h + 1) * nb],
                        in0=ex_h,
                        scalar1=rs[:, h : h + 1],
                        scalar2=None,
                        op0=OP.mult,
                    )
                return alpha

            def aggregate(xg, row, nb, alpha, psum, s2dt, ident, act_share):
                """psum[f, h*P+d] += sum_j alpha_h[d,j] * xg[d, j*row+f].
                One matmul per j with a diag-selector rhs; the diag pair is
                built on DVE (one op) or ACT (two ops) to balance engines."""
                xf = xg[:]
                for j in range(nb):
                    s2 = s2p.tile([P, 2 * P], s2dt, tag="s2" + ("b" if s2dt == BF16 else "f"))
                    if act_share and j % 2 == 1:
                        for h in range(H):
                            nc.scalar.activation(
                                out=s2[:, h * P : (h + 1) * P],
                                in_=ident[:],
                                func=AF.Copy,
                                scale=alpha[:, h * nb + j : h * nb + j + 1],
                            )
                    else:
                        s2v = s2[:].rearrange("p (h d) -> p h d", h=H)
                        id_b = _bcast_mid(ident[:], H)
                        al_b = bass.AP(
                            alpha.tensor,
                            alpha.offset + j,
                            [list(alpha.ap[0]), [nb, H], [0, P]],
                        )
                        nc.vector.tensor_tensor(out=s2v, in0=id_b, in1=al_b, op=OP.mult)
                    nc.tensor.matmul(
                        out=psum[:],
                        lhsT=xf[:, j * row : j * row + P],
                        rhs=s2[:],
                        start=(j == 0),
                        stop=(j == nb - 1),
                    )

            def allgather_maybe(b, own, tab, rowbytes_tensor_rows):
                """Issue the AllGather whose segment ends at block b+1."""
                for lo, hi in segs:
                    if b == hi - 1:
                        nc.gpsimd.collective_compute(
                            "AllGather", OP.bypass, replica_groups=[core_ids],
                            ins=[own[lo * P : min(hi * P, ND), :]],
                            outs=[
                                tab[
                                    NC * lo * P : NC * lo * P
                                    + NC * (min(hi * P, ND) - lo * P),
                                    :,
                                ]
                            ],
                        )

            # ================= Layer 1 + layer-2 table build =================
            for b in range(NBLK):
                nb = nch[b]
                base = sum(nch[:b])
                xg = xgp1.tile([P, nb * ROW1], BF16, tag="xg")
                src_ap = tab1e_in[base * P : (base + nb) * P, :].rearrange(
                    "(p j) r -> p (j r)", j=nb
                )
                nc.sync.dma_start(out=xg[:], in_=src_ap)
                alpha = attention_alphas(xg, ROW1, nb, ad1c[:, 2 * b : 2 * b + 2])
                psum1 = psp.tile([P, 2 * P], F32, tag="agg", space="PSUM")
                aggregate(xg, ROW1, nb, alpha, psum1, BF16, identb, act_share=True)
                agg_sb = pp.tile([P, 2 * P], BF16, tag="aggsb")
                nc.vector.tensor_copy(out=agg_sb[:], in_=psum1[:])
                psum_h1 = ps2.tile([P, 2 * P], F32, tag="h1", space="PSUM")
                for h in range(H):
                    nc.tensor.matmul(
                        out=psum_h1[:, h * P : (h + 1) * P],
                        lhsT=w1c[:, h * C1 : (h + 1) * C1],
                        rhs=agg_sb[:, h * P : (h + 1) * P],
                        start=True,
                        stop=True,
                    )
                h1T = pp.tile([P, 2 * P], BF16, tag="h1T")
                for h in range(H):
                    nc.vector.tensor_scalar(
                        out=h1T[:, h * P : (h + 1) * P],
                        in0=psum_h1[:, h * P : (h + 1) * P],
                        scalar1=b1c[:, h : h + 1],
                        scalar2=0.0,
                        op0=OP.add,
                        op1=OP.max,
                    )
                psum_h2 = ps2.tile([P, ROW2], F32, tag="h2", space="PSUM")
                nc.tensor.matmul(
                    out=psum_h2[:], lhsT=h1T[:, 0:P], rhs=w2e0[:], start=True, stop=False
                )
                nc.tensor.matmul(
                    out=psum_h2[:],
                    lhsT=h1T[:, P : 2 * P],
                    rhs=w2e1[:],
                    start=False,
                    stop=True,
                )
                h2sb = pp.tile([P, ROW2], F32R, tag="h2sb")
                nc.vector.tensor_copy(out=h2sb[:], in_=psum_h2[:])
                nc.vector.tensor_copy(
                    out=ad2c[:, 2 * b : 2 * b + 2],
                    in_=h2sb[:, H * C2 + H : H * C2 + 2 * H].bitcast(F32),
                )
                nc.sync.dma_start(
                    out=h2own[b * P : (b + 1) * P, :], in_=h2sb[:]
                )
                allgather_maybe(b, h2own, h2tab, ROW2)

            # ========================= Layer 2 =========================
            for b in range(NBLK):
                nb = nch[b]
                base = sum(nch[:b])
                xg = xgp2.tile([P, nb * ROW2], F32R, tag="xg")
                nc.sync.dma_start(
                    out=xg[:, 0:ROW2], in_=h2own[b * P : (b + 1) * P, :]
                )
                for j in range(1, nb):
                    nc.gpsimd.indirect_dma_start(
                        out=xg[:, j * ROW2 : (j + 1) * ROW2],
                        out_offset=None,
                        in_=h2tab[:, :],
                        in_offset=bass.IndirectOffsetOnAxis(
                            ap=srcidx2[:, base + j : base + j + 1], axis=0
                        ),
                    )
                alpha = attention_alphas(
                    xg, ROW2, nb, ad2c[:, 2 * b : 2 * b + 2]
                )
                psum2 = psp.tile([P, 2 * P], F32, tag="agg", space="PSUM")
                aggregate(xg, ROW2, nb, alpha, psum2, F32R, identf, act_share=False)
                agg2 = pp.tile([P, 2 * P], F32, tag="agg2")
                nc.vector.tensor_scalar(
                    out=agg2[:],
                    in0=psum2[:],
                    scalar1=b2c[:, 0:1],
                    scalar2=None,
                    op0=OP.add,
                )
                zsb = pp.tile([P, H * C2], F32, tag="zsb")
                for h in range(H):
                    pt = ps2.tile([P, C2], F32, tag="tp", space="PSUM")
                    nc.tensor.transpose(
                        out=pt[:],
                        in_=agg2[h * C2 : (h + 1) * C2, h * P : (h + 1) * P],
                        identity=ident64[h * C2 : (h + 1) * C2, :],
                    )
                    nc.vector.tensor_copy(
                        out=zsb[:, h * C2 : (h + 1) * C2], in_=pt[:]
                    )
                nc.sync.dma_start(
                    out=zown[b * P : (b + 1) * P, :], in_=zsb[:]
                )
                allgather_maybe(b, zown, zall, H * C2)

            # ========================= Decode =========================
            dec = cp.tile([P, DEC_CH], F32)
            for c in range(DEC_CH):
                zs = s2p.tile([P, H * C2], F32, tag="zs")
                nc.gpsimd.indirect_dma_start(
                    out=zs[:],
                    out_offset=None,
                    in_=zall[:, :],
                    in_offset=bass.IndirectOffsetOnAxis(
                        ap=posx[:, 2 * c : 2 * c + 1], axis=0
                    ),
                )
                zd = s2p.tile([P, H * C2], F32, tag="zd")
                nc.gpsimd.indirect_dma_start(
                    out=zd[:],
                    out_offset=None,
                    in_=zall[:, :],
                    in_offset=bass.IndirectOffsetOnAxis(
                        ap=posx[:, 2 * c + 1 : 2 * c + 2], axis=0
                    ),
                )
                prod = s2p.tile([P, H * C2], F32, tag="prod")
                nc.vector.tensor_tensor(out=prod[:], in0=zs[:], in1=zd[:], op=OP.mult)
                nc.vector.tensor_reduce(
                    out=dec[:, c : c + 1], in_=prod[:], axis=AX.X, op=OP.add
                )
            nc.sync.dma_start(out=dec_out[:], in_=dec[:])

    _split_waits(nc)
    return nc


def kernel(**inputs):
    x = np.asarray(inputs["x"], np.float32)
    ei = np.asarray(inputs["edge_index"], np.int64)
    pe = np.asarray(inputs["pos_edge_index"], np.int64)
    W1 = np.asarray(inputs["W1"], np.float32)
    a1s = np.asarray(inputs["a1_src"], np.float32)
    a1d = np.asarray(inputs["a1_dst"], np.float32)
    b1 = np.asarray(inputs["b1"], np.float32)
    W2 = np.asarray(inputs["W2"], np.float32)
    a2s = np.asarray(inputs["a2_src"], np.float32)
    a2d = np.asarray(inputs["a2_dst"], np.float32)
    b2 = np.asarray(inputs["b2"], np.float32)

    # -- edges with self loops, sorted by dst --
    src = np.concatenate([ei[0], np.arange(N, dtype=np.int64)]).astype(np.int32)
    dst = np.concatenate([ei[1], np.arange(N, dtype=np.int64)]).astype(np.int32)
    order = np.argsort(dst, kind="stable")
    ssrc = src[order]
    deg = np.bincount(dst, minlength=N).astype(np.int64)
    cum = np.zeros(N + 1, np.int64)
    np.cumsum(deg, out=cum[1:])

    # -- per-core degree-sorted slot schedule (uniform nch across cores) --
    slot_dst = np.full((NC, NBLK, P), -1, np.int64)  # global dst id, -1 dummy
    for c in range(NC):
        g = np.arange(c * ND, (c + 1) * ND, dtype=np.int64)
        perm = np.argsort(-deg[g], kind="stable")
        gs = g[perm]
        flat = slot_dst[c].reshape(-1)
        flat[: ND] = gs
    nch = []
    for b in range(NBLK):
        dm = 0
        for c in range(NC):
            sd = slot_dst[c, b]
            real = sd >= 0
            if real.any():
                dm = max(dm, int(deg[sd[real]].max()))
        nch.append(max(dm, 1))
    TC = int(sum(nch))

    # -- per-core slot->src table; column 0 is always the self loop --
    srcidx = np.full((NC, P, TC), PADG, np.int32)
    ad1t = np.zeros((NC, P, 2 * NBLK), np.float32)

    slotpos = np.zeros(N, np.int64)
    for c in range(NC):
        flat = slot_dst[c].reshape(-1)[:ND]
        slotpos[flat] = np.arange(ND)

    seg_lo = [0] + list(SPLITS)
    seg_hi = list(SPLITS) + [NBLK]
    seg_rows = [min(hi * P, ND) - lo * P for lo, hi in zip(seg_lo, seg_hi)]
    seg_row_lo = [lo * P for lo in seg_lo]
    seg_out_lo = np.cumsum([0] + [NC * r for r in seg_rows])[:-1]

    def rmap(g):
        """global node id -> row in the split-AllGather table layout."""
        g = np.asarray(g, np.int64)
        r = g // ND
        s_ = slotpos[np.clip(g, 0, N - 1)]
        pos = np.zeros_like(g)
        for k in range(len(seg_rows)):
            lo = seg_row_lo[k]
            hi = lo + seg_rows[k]
            m = (s_ >= lo) & (s_ < hi)
            pos = np.where(m, seg_out_lo[k] + r * seg_rows[k] + (s_ - lo), pos)
        return np.where(g >= N, g, pos).astype(np.int32)

    v1s = np.stack([W1[:, h * C1 : (h + 1) * C1] @ a1s[h] for h in range(H)], 1)
    v1d = np.stack([W1[:, h * C1 : (h + 1) * C1] @ a1d[h] for h in range(H)], 1)
    as1 = x @ v1s  # [N, H]
    ad1 = x @ v1d  # [N, H]

    base = 0
    for b in range(NBLK):
        nb = nch[b]
        for c in range(NC):
            sd = slot_dst[c, b]
            real = sd >= 0
            d = np.where(real, sd, 0)
            dg = deg[d] * real
            st = cum[d]
            for j in range(nb):
                m = dg > j
                if m.any():
                    srcidx[c, m, base + j] = ssrc[st[m] + j]
            # swap the self-loop edge into column 0
            cols = srcidx[c, :, base : base + nb]
            selfpos = np.argmax(cols == d[:, None], axis=1)
            has_self = (cols == d[:, None]).any(axis=1)
            assert bool(np.logical_or(~real, has_self).all()), "self loop missing"
            rowsel = np.arange(P)
            tmp = cols[rowsel, selfpos].copy()
            cols[rowsel, selfpos] = cols[:, 0]
            cols[:, 0] = np.where(real, tmp, PADG)
            ad1t[c, :, 2 * b : 2 * b + 2] = np.where(
                real[:, None], ad1[d], 0.0
            )
        base += nb
    srcidx2 = rmap(srcidx)

    # -- layer-1 edge-expanded stream tables (slot-ordered, per core, bf16) --
    xe = np.concatenate([x, as1], axis=1)  # [N, 130]
    padrow1 = np.zeros((1, ROW1), np.float32)
    padrow1[0, FIN:] = BIG_NEG
    xe = np.concatenate([xe, padrow1], axis=0).astype(ml_dtypes.bfloat16)
    tab1e = np.empty((NC, TC * P, ROW1), ml_dtypes.bfloat16)
    for c in range(NC):
        base = 0
        for b in range(NBLK):
            nb = nch[b]
            blk = xe[srcidx[c, :, base : base + nb].reshape(-1)]  # [(p j), ROW1]
            tab1e[c, base * P : (base + nb) * P] = blk
            base += nb

    # -- pos-edge decode tables --
    npc = EP // NC
    posidx = np.zeros((NC, P, 2 * DEC_CH), np.int32)
    for c in range(NC):
        s = pe[0, c * npc : (c + 1) * npc].astype(np.int32)
        d = pe[1, c * npc : (c + 1) * npc].astype(np.int32)
        sp = np.zeros(DEC_CH * P, np.int32)
        dp = np.zeros(DEC_CH * P, np.int32)
        sp[:npc] = rmap(s)
        dp[:npc] = rmap(d)
        posidx[c, :, 0::2] = sp.reshape(DEC_CH, P).T
        posidx[c, :, 1::2] = dp.reshape(DEC_CH, P).T

    # -- weights --
    v2s = np.stack([W2[:, h * C2 : (h + 1) * C2] @ a2s[h] for h in range(H)], 1)
    v2d = np.stack([W2[:, h * C2 : (h + 1) * C2] @ a2d[h] for h in range(H)], 1)
    w2e = np.concatenate([W2, v2s, v2d], axis=1).astype(ml_dtypes.bfloat16)
    b1col = b1.reshape(H, C1).T.astype(np.float32).copy()  # [128, 2]
    b2col = b2.reshape(P, 1).astype(np.float32).copy()
    identb = np.eye(P, dtype=ml_dtypes.bfloat16)
    identf = np.eye(P, dtype=np.float32)
    ident64 = np.tile(np.eye(C2, dtype=np.float32), (H, 1))
    padrow2 = np.zeros((2, ROW2), np.float32)
    padrow2[0, H * C2 : H * C2 + H] = BIG_NEG

    nc = _build_program(nch, TC)

    in_maps = []
    for c in range(NC):
        in_maps.append(
            {
                "tab1e": tab1e[c],
                "srcidx2": srcidx2[c],
                "ad1": ad1t[c],
                "posidx": posidx[c],
                "w1": W1.astype(ml_dtypes.bfloat16),
                "w2e": w2e,
                "b1col": b1col,
                "b2col": b2col,
                "identb": identb,
                "identf": identf,
                "ident64": ident64,
                "padrow2": padrow2,
            }
        )

    trace = bool(os.environ.get("KERNEL_TRACE"))
    res = run_bass_kernel_spmd(nc, in_maps, list(range(NC)), trace=trace)
    if trace:
        kernel.last_exec_ns = res.exec_time_ns
        kernel.last_mean_exec_ns = res.mean_exec_time_ns
    kernel.last_results = res.results

    out = np.empty(EP, np.float32)
    for c in range(NC):
        dec = res.results[c]["dec"]  # [P, DEC_CH]
        vals = dec.T.reshape(-1)[:npc]
        out[c * npc : (c + 1) * npc] = vals
    return out


# revision 11
# speedup vs baseline: 1.1928x; 1.1928x over previous
"""GAT link prediction on 8 TRN2 NeuronCores.

Sharding: dst nodes partitioned contiguously across 8 cores (6250 each).
Within a core, dsts are degree-sorted into 49 blocks of 128 (one dst per
SBUF partition); each block processes max-degree-in-block edge "chunks"
of 128 edges (slot (p, j) = j-th in-edge of the dst on partition p).

Layer 1 (bf16): the host edge-expands [x[src] | as1[src]] into a
slot-ordered bf16 DRAM table, so the kernel STREAMS it with one direct
HWDGE DMA per block (no per-edge indirection). Softmax over in-edges
runs per partition (dst) on DVE/ACT in f32; the alpha-weighted
aggregation is a PSUM-accumulated bf16 matmul with a diagonal selector
rhs (diag pairs built on DVE, every 3rd chunk's pair on ACT). Layer-1
output is transformed on-chip (W1, relu, W2ext, bf16) into the layer-2
gather table [h2 | a2_src | a2_dst] (f32r), all-gathered across cores
in 4 segments.

Layer 2 (f32r): per-edge rows are gathered from the all-gathered table
via indirect DMA (one SWDGE call per edge chunk; the self-loop chunk
comes from local h2own with a direct DMA — f32 rows keep the SWDGE call
at its cheapest). Decode gathers f32 z rows per positive edge and dots
them on DVE.
"""

import os
import numpy as np
import ml_dtypes

import concourse.bass as bass
import concourse.mybir as mybir
import concourse.tile as tile
from concourse.bass_utils import run_bass_kernel_spmd

NEG_SLOPE = 0.2
N = 50000
E = 800000
EP = 100000
H = 2
FIN = 128
C1 = 128   # per-head hidden (layer 1)
C2 = 64    # per-head out (layer 2)
NC = 8
P = 128
ND = N // NC          # dst nodes per core
NBLK = (ND + P - 1) // P   # 49
PADG = N              # gather-pad row (a_src = -1e30)
ROW1 = FIN + H        # 130: [x | as1_h0 | as1_h1]
ROW2 = H * C2 + 2 * H # 132: [h2 | as2_h0 | as2_h1 | ad2_h0 | ad2_h1]
DEC_CH = (EP // NC + P - 1) // P  # 98 decode chunks per core
SPLITS = (16, 32, 44)  # AllGather split points (blocks)

F32 = mybir.dt.float32
F32R = mybir.dt.float32r
BF16 = mybir.dt.bfloat16
I32 = mybir.dt.int32
AX = mybir.AxisListType
OP = mybir.AluOpType
AF = mybir.ActivationFunctionType
BIG_NEG = -1e30


def _split_waits(nc, max_waits=1):
    """This walrus build allows one sync-wait per instruction; move extra
    waits onto preceding same-engine NOPs (per-engine order preserved)."""
    total = 0
    for fn in nc.m.functions:
        for bb in fn.blocks:
            insts = bb.instructions
            i = 0
            while i < len(insts):
                inst = insts[i]
                si = inst.sync_info
                if si is not None and len(si.on_wait) > max_waits:
                    waits = list(si.on_wait)
                    keep = waits[-max_waits:]
                    extra = waits[:-max_waits]
                    inst.sync_info = mybir.SyncInfo(
                        on_wait=keep, on_update=list(si.on_update)
                    )
                    nops = []
                    for w in extra:
                        nop = mybir.InstNoOp(
                            name=nc.get_next_instruction_name(),
                            engine=inst.engine,
                            bass_nofuse=True,
                            sync_info=mybir.SyncInfo(on_wait=[w], on_update=[]),
                        )
                        nops.append(nop)
                        nc.register_instruction(nop, overwrite=True)
                    insts[i:i] = nops
                    i += len(nops)
                    total += len(nops)
                i += 1
    return total


def _bcast_mid(ap, n):
    """Insert a stride-0 middle dim: [p, k] view -> [p, n, k]."""
    pdim = ap.ap[0]
    rest = list(ap.ap[1:])
    return bass.AP(ap.tensor, ap.offset, [list(pdim), [0, n]] + [list(d) for d in rest])


def _seg_bounds(total_blocks, nd):
    """AllGather segment block ranges [(lo_blk, hi_blk, row_lo, row_hi)]."""
    segs = []
    prev = 0
    for s in SPLITS:
        segs.append((prev, s))
        prev = s
    segs.append((prev, total_blocks))
    return segs


def _build_program(nch, TC):
    core_ids = list(range(NC))
    nc = bass.Bass()

    # ---- kernel I/O ----
    tab1e_in = nc.dram_tensor("tab1e", [TC * P, ROW1], BF16, kind="ExternalInput")
    srcidx2_in = nc.dram_tensor("srcidx2", [P, TC], I32, kind="ExternalInput")
    ad1_in = nc.dram_tensor("ad1", [P, 2 * NBLK], F32, kind="ExternalInput")
    pos_in = nc.dram_tensor("posidx", [P, 2 * DEC_CH], I32, kind="ExternalInput")
    w1_in = nc.dram_tensor("w1", [FIN, H * C1], BF16, kind="ExternalInput")
    w2e_in = nc.dram_tensor("w2e", [H * C1, ROW2], BF16, kind="ExternalInput")
    b1_in = nc.dram_tensor("b1col", [P, H], F32, kind="ExternalInput")
    b2_in = nc.dram_tensor("b2col", [P, 1], F32, kind="ExternalInput")
    idb_in = nc.dram_tensor("identb", [P, P], BF16, kind="ExternalInput")
    idf_in = nc.dram_tensor("identf", [P, P], F32, kind="ExternalInput")
    id64_in = nc.dram_tensor("ident64", [P, C2], F32, kind="ExternalInput")
    pr2_in = nc.dram_tensor("padrow2", [2, ROW2], F32, kind="ExternalInput")
    dec_out = nc.dram_tensor("dec", [P, DEC_CH], F32, kind="ExternalOutput")

    # ---- internal DRAM ----
    h2own = nc.dram_tensor("h2own", [NBLK * P, ROW2], F32R)
    h2tab = nc.dram_tensor("h2tab", [N + 2, ROW2], F32R, addr_space="Shared")
    zown = nc.dram_tensor("zown", [NBLK * P, H * C2], F32)
    zall = nc.dram_tensor("zall", [N, H * C2], F32, addr_space="Shared")

    segs = _seg_bounds(NBLK, ND)

    with tile.TileContext(nc) as tc:
        with (
            tc.tile_pool(name="const", bufs=1) as cp,
            tc.tile_pool(name="xg1", bufs=3) as xgp1,
            tc.tile_pool(name="xg2", bufs=4) as xgp2,
            tc.tile_pool(name="att", bufs=2) as ap_,
            tc.tile_pool(name="s2", bufs=6) as s2p,
            tc.tile_pool(name="post", bufs=2) as pp,
            tc.tile_pool(name="psum", bufs=2, space="PSUM") as psp,
            tc.tile_pool(name="psum2", bufs=2, space="PSUM") as ps2,
        ):
            # ---- constants to SBUF ----
            srcidx2 = cp.tile([P, TC], I32)
            nc.sync.dma_start(out=srcidx2[:], in_=srcidx2_in[:])
            ad1c = cp.tile([P, 2 * NBLK], F32)
            nc.sync.dma_start(out=ad1c[:], in_=ad1_in[:])
            posx = cp.tile([P, 2 * DEC_CH], I32)
            nc.sync.dma_start(out=posx[:], in_=pos_in[:])
            w1c = cp.tile([P, H * C1], BF16)
            nc.sync.dma_start(out=w1c[:], in_=w1_in[:])
            w2e0 = cp.tile([P, ROW2], BF16)
            nc.sync.dma_start(out=w2e0[:], in_=w2e_in[0:P, :])
            w2e1 = cp.tile([P, ROW2], BF16)
            nc.sync.dma_start(out=w2e1[:], in_=w2e_in[P : 2 * P, :])
            b1c = cp.tile([P, H], F32)
            nc.sync.dma_start(out=b1c[:], in_=b1_in[:])
            b2c = cp.tile([P, 1], F32)
            nc.sync.dma_start(out=b2c[:], in_=b2_in[:])
            identb = cp.tile([P, P], BF16)
            nc.sync.dma_start(out=identb[:], in_=idb_in[:])
            identf = cp.tile([P, P], F32)
            nc.sync.dma_start(out=identf[:], in_=idf_in[:])
            ident64 = cp.tile([P, C2], F32)
            nc.sync.dma_start(out=ident64[:], in_=id64_in[:])
            ad2c = cp.tile([P, 2 * NBLK], F32)
            pr2s = cp.tile([2, ROW2], F32)
            nc.sync.dma_start(out=pr2s[:], in_=pr2_in[:])
            nc.sync.dma_start(out=h2tab[N : N + 2, :].bitcast(F32), in_=pr2s[:])

            def attention_alphas(xg, row, nb, ad_ap):
                """xg: [P, nb*row] rows; a_src at cols FIN.. or H*C2..;
                returns alpha tile [P, 2*nb] (head-major, dtype adt)."""
                xv = xg[:].rearrange("p (j r) -> p j r", r=row)
                if xg.dtype == F32R:
                    xv = xv.bitcast(F32)
                as_ap = xv[:, :, FIN if row == ROW1 else H * C2 :][:, :, 0:H]
                ex = ap_.tile([P, 2 * nb], F32, tag="ex")
                exv = ex[:].rearrange("p (j h) -> p j h", h=H)
                nc.vector.tensor_tensor(
                    out=exv, in0=as_ap, in1=_bcast_mid(ad_ap, nb), op=OP.add
                )
                # leaky relu fused on DVE: ex = max(ex * slope, ex); exp on ACT
                nc.vector.scalar_tensor_tensor(
                    out=ex[:], in0=ex[:], scalar=NEG_SLOPE, in1=ex[:],
                    op0=OP.mult, op1=OP.max,
                )
                nc.scalar.activation(out=ex[:], in_=ex[:], func=AF.Exp)
                s = ap_.tile([P, H], F32, tag="s")
                ex_hj = bass.AP(
                    ex.tensor, ex.offset, [list(ex.ap[0]), [1, H], [H, nb]]
                )
                nc.vector.tensor_reduce(out=s[:], in_=ex_hj, axis=AX.X, op=OP.add)
                nc.vector.tensor_scalar(
                    out=s[:], in0=s[:], scalar1=1e-30, scalar2=None, op0=OP.add
                )
                rs = ap_.tile([P, H], F32, tag="rs")
                nc.vector.reciprocal(out=rs[:], in_=s[:])
                alpha = ap_.tile([P, 2 * nb], F32, tag="alpha")
                for h in range(H):
                    ex_h = bass.AP(
                        ex.tensor, ex.offset + h, [list(ex.ap[0]), [H, nb]]
                    )
                    nc.vector.tensor_scalar(
                        out=alpha[:, h * nb : (h + 1) * nb],
                        in0=ex_h,
                        scalar1=rs[:, h : h + 1],
                        scalar2=None,
                        op0=OP.mult,
                    )
                return alpha

            def aggregate(xg, row, nb, alpha, psum, s2dt, ident, act_share):
                """psum[f, h*P+d] += sum_j alpha_h[d,j] * xg[d, j*row+f].
                One matmul per j with a diag-selector rhs; the diag pair is
                built on DVE (one op) or ACT (two ops) to balance engines."""
                xf = xg[:]
                for j in range(nb):
                    s2 = s2p.tile([P, 2 * P], s2dt, tag="s2" + ("b" if s2dt == BF16 else "f"))
                    if act_share and j % 3 == 2:
                        for h in range(H):
                            nc.scalar.activation(
                                out=s2[:, h * P : (h + 1) * P],
                                in_=ident[:],
                                func=AF.Copy,
                                scale=alpha[:, h * nb + j : h * nb + j + 1],
                            )
                    else:
                        s2v = s2[:].rearrange("p (h d) -> p h d", h=H)
                        id_b = _bcast_mid(ident[:], H)
                        al_b = bass.AP(
                            alpha.tensor,
                            alpha.offset + j,
                            [list(alpha.ap[0]), [nb, H], [0, P]],
                        )
                        nc.vector.tensor_tensor(out=s2v, in0=id_b, in1=al_b, op=OP.mult)
                    nc.tensor.matmul(
                        out=psum[:],
                        lhsT=xf[:, j * row : j * row + P],
                        rhs=s2[:],
                        start=(j == 0),
                        stop=(j == nb - 1),
                    )

            def allgather_maybe(b, own, tab, rowbytes_tensor_rows):
                """Issue the AllGather whose segment ends at block b+1."""
                for lo, hi in segs:
                    if b == hi - 1:
                        nc.gpsimd.collective_compute(
                            "AllGather", OP.bypass, replica_groups=[core_ids],
                            ins=[own[lo * P : min(hi * P, ND), :]],
                            outs=[
                                tab[
                                    NC * lo * P : NC * lo * P
                                    + NC * (min(hi * P, ND) - lo * P),
                                    :,
                                ]
                            ],
                        )

            # ================= Layer 1 + layer-2 table build =================
            for b in range(NBLK):
                nb = nch[b]
                base = sum(nch[:b])
                xg = xgp1.tile([P, nb * ROW1], BF16, tag="xg")
                src_ap = tab1e_in[base * P : (base + nb) * P, :].rearrange(
                    "(p j) r -> p (j r)", j=nb
                )
                nc.sync.dma_start(out=xg[:], in_=src_ap)
                alpha = attention_alphas(xg, ROW1, nb, ad1c[:, 2 * b : 2 * b + 2])
                psum1 = psp.tile([P, 2 * P], F32, tag="agg", space="PSUM")
                aggregate(xg, ROW1, nb, alpha, psum1, BF16, identb, act_share=True)
                agg_sb = pp.tile([P, 2 * P], BF16, tag="aggsb")
                nc.vector.tensor_copy(out=agg_sb[:], in_=psum1[:])
                psum_h1 = ps2.tile([P, 2 * P], F32, tag="h1", space="PSUM")
                for h in range(H):
                    nc.tensor.matmul(
                        out=psum_h1[:, h * P : (h + 1) * P],
                        lhsT=w1c[:, h * C1 : (h + 1) * C1],
                        rhs=agg_sb[:, h * P : (h + 1) * P],
                        start=True,
                        stop=True,
                    )
                h1T = pp.tile([P, 2 * P], BF16, tag="h1T")
                for h in range(H):
                    nc.vector.tensor_scalar(
                        out=h1T[:, h * P : (h + 1) * P],
                        in0=psum_h1[:, h * P : (h + 1) * P],
                        scalar1=b1c[:, h : h + 1],
                        scalar2=0.0,
                        op0=OP.add,
                        op1=OP.max,
                    )
                psum_h2 = ps2.tile([P, ROW2], F32, tag="h2", space="PSUM")
                nc.tensor.matmul(
                    out=psum_h2[:], lhsT=h1T[:, 0:P], rhs=w2e0[:], start=True, stop=False
                )
                nc.tensor.matmul(
                    out=psum_h2[:],
                    lhsT=h1T[:, P : 2 * P],
                    rhs=w2e1[:],
                    start=False,
                    stop=True,
                )
                h2sb = pp.tile([P, ROW2], F32R, tag="h2sb")
                nc.vector.tensor_copy(out=h2sb[:], in_=psum_h2[:])
                nc.vector.tensor_copy(
                    out=ad2c[:, 2 * b : 2 * b + 2],
                    in_=h2sb[:, H * C2 + H : H * C2 + 2 * H].bitcast(F32),
                )
                nc.sync.dma_start(
                    out=h2own[b * P : (b + 1) * P, :], in_=h2sb[:]
                )
                allgather_maybe(b, h2own, h2tab, ROW2)

            # ========================= Layer 2 =========================
            for b in range(NBLK):
                nb = nch[b]
                base = sum(nch[:b])
                xg = xgp2.tile([P, nb * ROW2], F32R, tag="xg")
                nc.sync.dma_start(
                    out=xg[:, 0:ROW2], in_=h2own[b * P : (b + 1) * P, :]
                )
                for j in range(1, nb):
                    nc.gpsimd.indirect_dma_start(
                        out=xg[:, j * ROW2 : (j + 1) * ROW2],
                        out_offset=None,
                        in_=h2tab[:, :],
                        in_offset=bass.IndirectOffsetOnAxis(
                            ap=srcidx2[:, base + j : base + j + 1], axis=0
                        ),
                    )
                alpha = attention_alphas(
                    xg, ROW2, nb, ad2c[:, 2 * b : 2 * b + 2]
                )
                psum2 = psp.tile([P, 2 * P], F32, tag="agg", space="PSUM")
                aggregate(xg, ROW2, nb, alpha, psum2, F32R, identf, act_share=False)
                agg2 = pp.tile([P, 2 * P], F32, tag="agg2")
                nc.vector.tensor_scalar(
                    out=agg2[:],
                    in0=psum2[:],
                    scalar1=b2c[:, 0:1],
                    scalar2=None,
                    op0=OP.add,
                )
                zsb = pp.tile([P, H * C2], F32, tag="zsb")
                for h in range(H):
                    pt = ps2.tile([P, C2], F32, tag="tp", space="PSUM")
                    nc.tensor.transpose(
                        out=pt[:],
                        in_=agg2[h * C2 : (h + 1) * C2, h * P : (h + 1) * P],
                        identity=ident64[h * C2 : (h + 1) * C2, :],
                    )
                    nc.vector.tensor_copy(
                        out=zsb[:, h * C2 : (h + 1) * C2], in_=pt[:]
                    )
                nc.sync.dma_start(
                    out=zown[b * P : (b + 1) * P, :], in_=zsb[:]
                )
                allgather_maybe(b, zown, zall, H * C2)

            # ========================= Decode =========================
            dec = cp.tile([P, DEC_CH], F32)
            for c in range(DEC_CH):
                zs = s2p.tile([P, H * C2], F32, tag="zs")
                nc.gpsimd.indirect_dma_start(
                    out=zs[:],
                    out_offset=None,
                    in_=zall[:, :],
                    in_offset=bass.IndirectOffsetOnAxis(
                        ap=posx[:, 2 * c : 2 * c + 1], axis=0
                    ),
                )
                zd = s2p.tile([P, H * C2], F32, tag="zd")
                nc.gpsimd.indirect_dma_start(
                    out=zd[:],
                    out_offset=None,
                    in_=zall[:, :],
                    in_offset=bass.IndirectOffsetOnAxis(
                        ap=posx[:, 2 * c + 1 : 2 * c + 2], axis=0
                    ),
                )
                prod = s2p.tile([P, H * C2], F32, tag="prod")
                nc.vector.tensor_tensor(out=prod[:], in0=zs[:], in1=zd[:], op=OP.mult)
                nc.vector.tensor_reduce(
                    out=dec[:, c : c + 1], in_=prod[:], axis=AX.X, op=OP.add
                )
            nc.sync.dma_start(out=dec_out[:], in_=dec[:])

    _split_waits(nc)
    return nc


def kernel(**inputs):
    x = np.asarray(inputs["x"], np.float32)
    ei = np.asarray(inputs["edge_index"], np.int64)
    pe = np.asarray(inputs["pos_edge_index"], np.int64)
    W1 = np.asarray(inputs["W1"], np.float32)
    a1s = np.asarray(inputs["a1_src"], np.float32)
    a1d = np.asarray(inputs["a1_dst"], np.float32)
    b1 = np.asarray(inputs["b1"], np.float32)
    W2 = np.asarray(inputs["W2"], np.float32)
    a2s = np.asarray(inputs["a2_src"], np.float32)
    a2d = np.asarray(inputs["a2_dst"], np.float32)
    b2 = np.asarray(inputs["b2"], np.float32)

    # -- edges with self loops, sorted by dst --
    src = np.concatenate([ei[0], np.arange(N, dtype=np.int64)]).astype(np.int32)
    dst = np.concatenate([ei[1], np.arange(N, dtype=np.int64)]).astype(np.int32)
    order = np.argsort(dst, kind="stable")
    ssrc = src[order]
    deg = np.bincount(dst, minlength=N).astype(np.int64)
    cum = np.zeros(N + 1, np.int64)
    np.cumsum(deg, out=cum[1:])

    # -- per-core degree-sorted slot schedule (uniform nch across cores) --
    slot_dst = np.full((NC, NBLK, P), -1, np.int64)  # global dst id, -1 dummy
    for c in range(NC):
        g = np.arange(c * ND, (c + 1) * ND, dtype=np.int64)
        perm = np.argsort(-deg[g], kind="stable")
        gs = g[perm]
        flat = slot_dst[c].reshape(-1)
        flat[: ND] = gs
    nch = []
    for b in range(NBLK):
        dm = 0
        for c in range(NC):
            sd = slot_dst[c, b]
            real = sd >= 0
            if real.any():
                dm = max(dm, int(deg[sd[real]].max()))
        nch.append(max(dm, 1))
    TC = int(sum(nch))

    # -- per-core slot->src table; column 0 is always the self loop --
    srcidx = np.full((NC, P, TC), PADG, np.int32)
    ad1t = np.zeros((NC, P, 2 * NBLK), np.float32)

    slotpos = np.zeros(N, np.int64)
    for c in range(NC):
        flat = slot_dst[c].reshape(-1)[:ND]
        slotpos[flat] = np.arange(ND)

    seg_lo = [0] + list(SPLITS)
    seg_hi = list(SPLITS) + [NBLK]
    seg_rows = [min(hi * P, ND) - lo * P for lo, hi in zip(seg_lo, seg_hi)]
    seg_row_lo = [lo * P for lo in seg_lo]
    seg_out_lo = np.cumsum([0] + [NC * r for r in seg_rows])[:-1]

    def rmap(g):
        """global node id -> row in the split-AllGather table layout."""
        g = np.asarray(g, np.int64)
        r = g // ND
        s_ = slotpos[np.clip(g, 0, N - 1)]
        pos = np.zeros_like(g)
        for k in range(len(seg_rows)):
            lo = seg_row_lo[k]
            hi = lo + seg_rows[k]
            m = (s_ >= lo) & (s_ < hi)
            pos = np.where(m, seg_out_lo[k] + r * seg_rows[k] + (s_ - lo), pos)
        return np.where(g >= N, g, pos).astype(np.int32)

    v1s = np.stack([W1[:, h * C1 : (h + 1) * C1] @ a1s[h] for h in range(H)], 1)
    v1d = np.stack([W1[:, h * C1 : (h + 1) * C1] @ a1d[h] for h in range(H)], 1)
    as1 = x @ v1s  # [N, H]
    ad1 = x @ v1d  # [N, H]

    base = 0
    for b in range(NBLK):
        nb = nch[b]
        for c in range(NC):
            sd = slot_dst[c, b]
            real = sd >= 0
            d = np.where(real, sd, 0)
            dg = deg[d] * real
            st = cum[d]
            for j in range(nb):
                m = dg > j
                if m.any():
                    srcidx[c, m, base + j] = ssrc[st[m] + j]
            # swap the self-loop edge into column 0
            cols = srcidx[c, :, base : base + nb]
            selfpos = np.argmax(cols == d[:, None], axis=1)
            has_self = (cols == d[:, None]).any(axis=1)
            assert bool(np.logical_or(~real, has_self).all()), "self loop missing"
            rowsel = np.arange(P)
            tmp = cols[rowsel, selfpos].copy()
            cols[rowsel, selfpos] = cols[:, 0]
            cols[:, 0] = np.where(real, tmp, PADG)
            ad1t[c, :, 2 * b : 2 * b + 2] = np.where(
                real[:, None], ad1[d], 0.0
            )
        base += nb
    srcidx2 = rmap(srcidx)

    # -- layer-1 edge-expanded stream tables (slot-ordered, per core, bf16) --
    xe = np.concatenate([x, as1], axis=1)  # [N, 130]
    padrow1 = np.zeros((1, ROW1), np.float32)
    padrow1[0, FIN:] = BIG_NEG
    xe = np.concatenate([xe, padrow1], axis=0).astype(ml_dtypes.bfloat16)
    tab1e = np.empty((NC, TC * P, ROW1), ml_dtypes.bfloat16)
    for c in range(NC):
        base = 0
        for b in range(NBLK):
            nb = nch[b]
            blk = xe[srcidx[c, :, base : base + nb].reshape(-1)]  # [(p j), ROW1]
            tab1e[c, base * P : (base + nb) * P] = blk
            base += nb

    # -- pos-edge decode tables --
    npc = EP // NC
    posidx = np.zeros((NC, P, 2 * DEC_CH), np.int32)
    for c in range(NC):
        s = pe[0, c * npc : (c + 1) * npc].astype(np.int32)
        d = pe[1, c * npc : (c + 1) * npc].astype(np.int32)
        sp = np.zeros(DEC_CH * P, np.int32)
        dp = np.zeros(DEC_CH * P, np.int32)
        sp[:npc] = rmap(s)
        dp[:npc] = rmap(d)
        posidx[c, :, 0::2] = sp.reshape(DEC_CH, P).T
        posidx[c, :, 1::2] = dp.reshape(DEC_CH, P).T

    # -- weights --
    v2s = np.stack([W2[:, h * C2 : (h + 1) * C2] @ a2s[h] for h in range(H)], 1)
    v2d = np.stack([W2[:, h * C2 : (h + 1) * C2] @ a2d[h] for h in range(H)], 1)
    w2e = np.concatenate([W2, v2s, v2d], axis=1).astype(ml_dtypes.bfloat16)
    b1col = b1.reshape(H, C1).T.astype(np.float32).copy()  # [128, 2]
    b2col = b2.reshape(P, 1).astype(np.float32).copy()
    identb = np.eye(P, dtype=ml_dtypes.bfloat16)
    identf = np.eye(P, dtype=np.float32)
    ident64 = np.tile(np.eye(C2, dtype=np.float32), (H, 1))
    padrow2 = np.zeros((2, ROW2), np.float32)
    padrow2[0, H * C2 : H * C2 + H] = BIG_NEG

    nc = _build_program(nch, TC)

    in_maps = []
    for c in range(NC):
        in_maps.append(
            {
                "tab1e": tab1e[c],
                "srcidx2": srcidx2[c],
                "ad1": ad1t[c],
                "posidx": posidx[c],
                "w1": W1.astype(ml_dtypes.bfloat16),
                "w2e": w2e,
                "b1col": b1col,
                "b2col": b2col,
                "identb": identb,
                "identf": identf,
                "ident64": ident64,
                "padrow2": padrow2,
            }
        )

    trace = bool(os.environ.get("KERNEL_TRACE"))
    res = run_bass_kernel_spmd(nc, in_maps, list(range(NC)), trace=trace)
    if trace:
        kernel.last_exec_ns = res.exec_time_ns
        kernel.last_mean_exec_ns = res.mean_exec_time_ns
    kernel.last_results = res.results

    out = np.empty(EP, np.float32)
    for c in range(NC):
        dec = res.results[c]["dec"]  # [P, DEC_CH]
        vals = dec.T.reshape(-1)[:npc]
        out[c * npc : (c + 1) * npc] = vals
    return out
